# revision 34
# baseline (speedup 1.0000x reference)
"""Mixtral-style GQA attention block, tensor-parallel over 8 NeuronCores.

Sharding: core i owns q heads 4i..4i+3 and kv head i (GQA group == 4, so the
kv head's whole group lives on one core).  w_qkv is column-sharded by head.
o_proj is token-sharded via contraction-split: each core multiplies its own
attention outputs (kept resident in SBUF) against its 512-column slice of
w_o, producing a full-width partial [4096, T]; a bf16 ReduceScatter per
token-quarter sums the partials and leaves core i with the 512-row output
slice it owns.

Emission order interleaves the three stages so the PE never waits on a
phase boundary and the ReduceScatter chain drains under attention compute:
  qkv(0) qkv(1) attn(0) qkv(2) attn(1) qkv(3)
  attn(2) oproj(0)+RS0 attn(3) oproj(1)+RS1 oproj(2)+RS2 oproj(3)+RS3

All matmuls run in bf16 (fp32 PSUM accumulation); softmax runs in fp32
without max-subtraction (scores are ~N(0,1) by construction, exp cannot
overflow).
"""

import os
import numpy as np
import ml_dtypes
from contextlib import ExitStack

import concourse.bass as bass
import concourse.bass_isa as bass_isa
import concourse.mybir as mybir
import concourse.tile as tile
from concourse import bacc
from concourse.bass_utils import run_bass_kernel_spmd

_NO_RS = bool(int(os.environ.get("K_NO_RS", "0")))   # ablation only

P = 128
HID = 4096
D = 128
QH = 4                      # local q heads per core
NB = 6                      # projection M-blocks: q0..q3, k, v
KC = HID // P               # contraction chunks over hidden dim
N_CORES = 8
SCALE = float(D) ** -0.5
NEG = -1.0e30

dt = mybir.dt
bf16 = ml_dtypes.bfloat16

F32 = dt.float32
BF16 = dt.bfloat16


def build_nc(t_len=2048, phases=3, reps=1):
    TCH = t_len // P            # token chunks
    TB = 512                    # qkv projection token-block width
    TQH = max(t_len // 4, P)    # attention tq-block width
    R = min(512, TQH)           # psum accumulation region width
    NR = TQH // R
    WQ = NB * P                 # 768
    WO = QH * P                 # 512
    QT = min(512, t_len)        # o_proj token-tile width
    QT = 512
    nqt = t_len // QT
    MB = HID // P               # o_proj hidden-out blocks (32)
    n_tq = t_len // TQH

    nc = bacc.Bacc("TRN2", target_bir_lowering=False, debug=False,
                   num_devices=N_CORES)

    hiddenT = nc.dram_tensor("hiddenT", [HID, t_len], BF16, kind="ExternalInput").ap()
    wqkvT = nc.dram_tensor("wqkvT", [HID, WQ], BF16, kind="ExternalInput").ap()
    woT = nc.dram_tensor("woT", [WO, HID], BF16, kind="ExternalInput").ap()
    cos2 = nc.dram_tensor("cos2", [P, t_len], F32, kind="ExternalInput").ap()
    sin2 = nc.dram_tensor("sin2", [P, t_len], F32, kind="ExternalInput").ap()
    maskd = nc.dram_tensor("maskd", [P, P], F32, kind="ExternalInput").ap()
    outp = nc.dram_tensor("outp", [WO, t_len], BF16, kind="ExternalOutput").ap()

    with tile.TileContext(nc) as tc:
        with ExitStack() as whole:
            persist = whole.enter_context(tc.tile_pool(name="persist", bufs=1))
            dram = whole.enter_context(tc.tile_pool(name="dram", bufs=1, space="DRAM"))

            # ---- constants ----
            mask_sb = persist.tile([P, P], F32, tag="mask")
            # full ones matrix: the l matmul then leaves the row-sum
            # broadcast across all 128 output partitions directly
            ones_sb = persist.tile([P, P], BF16, tag="ones")
            nc.gpsimd.dma_start(mask_sb[:], maskd[:])
            nc.vector.memset(ones_sb[:], 1.0)

            # ---- persistent activations ----
            qk_sb = [persist.tile([P, t_len], BF16, tag=f"qk{mb}", name=f"qk{mb}")
                     for mb in range(5)]
            v_sb = persist.tile([P, TCH * P], BF16, tag="v", name="v_sb")
            # attention outputs overwrite the q tiles in place: block `half`
            # of head h reads qT[h][:, tq0:tq0+TQH] exactly once (its score
            # matmuls), after which the region is dead -- so the normalized
            # attention output for that block can alias it.  o_proj then
            # reads its rhs straight from these tiles.
            attn_sb = qk_sb[:QH]

            for rep in range(reps):
              partial = [dram.tile([HID, QT], BF16, tag=f"partial{rep}_{s}",
                                   name=f"partial{rep}_{s}")
                         for s in range(nqt)]
              rs_out = [dram.tile([WO, QT], BF16, tag=f"rs{rep}_{s}",
                                  name=f"rs{rep}_{s}")
                        for s in range(nqt)]

              with ExitStack() as body:
                # ---------- attention pools (live for the whole rep) ----------
                p_pool = body.enter_context(tc.tile_pool(name="pstrips", bufs=20))
                st_pool = body.enter_context(
                    tc.tile_pool(name="st_psum", bufs=2, space="PSUM"))
                out_ps_pool = body.enter_context(
                    tc.tile_pool(name="attn_out_psum", bufs=1, space="PSUM"))
                l_ps_pool = body.enter_context(
                    tc.tile_pool(name="l_psum", bufs=1, space="PSUM"))
                misc = body.enter_context(tc.tile_pool(name="attn_misc", bufs=2))

                # ---------- phase-1 pools (closed after last qkv block) ------
                ph1 = ExitStack()
                consts = ph1.enter_context(tc.tile_pool(name="consts", bufs=1))
                hid_pool = ph1.enter_context(tc.tile_pool(name="hid", bufs=2))
                wq_pool = ph1.enter_context(tc.tile_pool(name="wq", bufs=1))
                ps_pool = ph1.enter_context(
                    tc.tile_pool(name="proj_psum", bufs=4, space="PSUM"))
                stage = ph1.enter_context(tc.tile_pool(name="stage", bufs=6))
                vpool = ph1.enter_context(tc.tile_pool(name="vstage", bufs=2))

                cos2_sb = consts.tile([P, t_len], F32, tag="cos2")
                sin2_sb = consts.tile([P, t_len], F32, tag="sin2")
                nc.gpsimd.dma_start(cos2_sb[:], cos2[:])
                nc.gpsimd.dma_start(sin2_sb[:], sin2[:])

                # cache all of wqkvT in SBUF: chunk c at [:, c*WQ:(c+1)*WQ]
                wqkv_sb = wq_pool.tile([P, KC * WQ], BF16, tag="wqkv")
                wqkv_src = wqkvT.rearrange("(c p) w -> p c w", p=P)
                wqkv_dst = wqkv_sb.rearrange("p (c w) -> p c w", w=WQ)
                # fine-grained leading chunks so the first matmuls start
                # early; the very first matmul (pair (4,5), c=0) only reads
                # the k/v columns of chunk 0, so those 256 columns go first
                nc.scalar.dma_start(wqkv_dst[:, 0:1, 4 * P:6 * P],
                                    wqkv_src[:, 0:1, 4 * P:6 * P])
                nc.scalar.dma_start(wqkv_dst[:, 0:1, 0:4 * P],
                                    wqkv_src[:, 0:1, 0:4 * P])
                for a, b in [(1, 2), (2, 3), (3, 4), (4, 8),
                             (8, 16), (16, 24), (24, 32)]:
                    nc.scalar.dma_start(wqkv_dst[:, a:b, :],
                                        wqkv_src[:, a:b, :])

                hid_src = hiddenT.rearrange("(c p) t -> p c t", p=P)

                def emit_qkv_block(tb):
                    ta = tb * TB
                    hid_b = hid_pool.tile([P, KC * TB], BF16, tag="hidb",
                                          name=f"hid{tb}")
                    hb_dst = hid_b.rearrange("p (c t) -> p c t", t=TB)
                    nc.sync.dma_start(hb_dst[:, 0:1, :],
                                      hid_src[:, 0:1, ta:ta + TB])
                    nc.sync.dma_start(hb_dst[:, 1:8, :],
                                      hid_src[:, 1:8, ta:ta + TB])
                    for c0 in range(8, KC, 8):
                        nc.sync.dma_start(hb_dst[:, c0:c0 + 8, :],
                                          hid_src[:, c0:c0 + 8, ta:ta + TB])
                    for pair, mbs in enumerate([(4, 5), (0, 1), (2, 3)]):
                        psums = []
                        for u in range(2):
                            pt = ps_pool.tile([P, TB], F32, tag="pj",
                                              name=f"pj{tb}_{pair}_{u}")
                            psums.append(pt)
                        for c in range(KC):
                            for u in range(2):
                                mb = mbs[u]
                                lhsT = wqkv_sb[:, c * WQ + mb * P:
                                               c * WQ + (mb + 1) * P]
                                nc.tensor.matmul(
                                    psums[u][:], lhsT=lhsT,
                                    rhs=hid_b[:, c * TB:(c + 1) * TB],
                                    start=(c == 0), stop=(c == KC - 1))
                        for u in range(2):
                            mb = mbs[u]
                            if mb < 5:
                                # rope: qk[d] = raw[d]*cos2[d]
                                #             + raw[(d+64)%128]*sin2[d]
                                raw = stage.tile([P, TB], F32, tag="stg",
                                                 name="raw")
                                nc.vector.tensor_copy(raw[:], psums[u][:])
                                rot = stage.tile([P, TB], F32, tag="stg",
                                                 name="rot")
                                nc.gpsimd.dma_start(rot[0:64, :], raw[64:128, :])
                                nc.gpsimd.dma_start(rot[64:128, :], raw[0:64, :])
                                t1 = stage.tile([P, TB], F32, tag="stg",
                                                name="t1")
                                nc.vector.tensor_mul(t1[:], raw[:],
                                                     cos2_sb[:, ta:ta + TB])
                                t2 = stage.tile([P, TB], F32, tag="stg",
                                                name="t2")
                                nc.vector.tensor_mul(t2[:], rot[:],
                                                     sin2_sb[:, ta:ta + TB])
                                nc.vector.tensor_add(qk_sb[mb][:, ta:ta + TB],
                                                     t1[:], t2[:])
                            else:
                                # v: evacuate bf16 [d, t], DMA-transpose each
                                # [d, tk] chunk into [tk, d]
                                vstg = vpool.tile([P, TB], BF16, tag="vstg",
                                                  name="vstg")
                                nc.vector.tensor_copy(vstg[:], psums[u][:])
                                for ct in range(TB // P):
                                    gc = tb * (TB // P) + ct
                                    nc.sync.dma_start_transpose(
                                        v_sb[:, gc * P:(gc + 1) * P],
                                        vstg[:, ct * P:(ct + 1) * P])

                kT = qk_sb[4]

                def emit_attn_block(half):
                    for h in range(QH):
                        qT = qk_sb[h]
                        tq0 = TQH * half
                        ncv = (tq0 + TQH) // P    # contributing tk chunks
                        # ---- pass A: scores + exp -> P strips ----
                        strips = []
                        for c in range(ncv):
                            off = max(tq0, P * c)
                            w = tq0 + TQH - off
                            st = st_pool.tile([P, TQH], F32, tag="st",
                                              name=f"st{h}_{half}_{c}")
                            for s0 in range(0, w, 512):
                                s1 = min(s0 + 512, w)
                                nc.tensor.matmul(
                                    st[:, s0:s1],
                                    lhsT=kT[:, c * P:(c + 1) * P],
                                    rhs=qT[:, off + s0:off + s1],
                                    start=True, stop=True)
                            if P * c >= tq0:
                                # strip starts on the diagonal: mask tq<tk
                                nc.vector.tensor_add(
                                    st[:, 0:P], st[:, 0:P], mask_sb[:])
                            pt = p_pool.tile([P, TQH], BF16, tag="p",
                                             name=f"p{h}_{half}_{c}")
                            nc.scalar.activation(
                                pt[:, 0:w], st[:, 0:w],
                                mybir.ActivationFunctionType.Exp, scale=SCALE)
                            strips.append((pt, off, w))
                        # ---- pass B: PV and row-sums, region-wise ----
                        out_ps = out_ps_pool.tile([P, TQH], F32, tag="op",
                                                  name="out_ps")
                        l_ps = l_ps_pool.tile([P, TQH], F32, tag="lp",
                                              name="l_ps")
                        for r in range(NR):
                            r0 = tq0 + R * r
                            cmax = (r0 + R - 1) // P
                            for c in range(cmax + 1):
                                pt, off, w = strips[c]
                                a = max(0, r0 - off)
                                b = max(0, off - r0)
                                wr = min(off + w, r0 + R) - max(off, r0)
                                dst0 = R * r + b
                                for s0 in range(0, wr, 512):
                                    s1 = min(s0 + 512, wr)
                                    nc.tensor.matmul(
                                        out_ps[:, dst0 + s0:dst0 + s1],
                                        lhsT=v_sb[:, c * P:(c + 1) * P],
                                        rhs=pt[:, a + s0:a + s1],
                                        start=(c == 0), stop=(c == cmax))
                                    nc.tensor.matmul(
                                        l_ps[:, dst0 + s0:dst0 + s1],
                                        lhsT=ones_sb[:],
                                        rhs=pt[:, a + s0:a + s1],
                                        start=(c == 0), stop=(c == cmax))
                        # ---- epilogue: normalize by row-sums ----
                        oraw = misc.tile([P, TQH], F32, tag="oraw", name="oraw")
                        nc.vector.tensor_copy(oraw[:], out_ps[:])
                        inv_t = misc.tile([P, TQH], F32, tag="inv", name="inv_t")
                        nc.vector.reciprocal(inv_t[:], l_ps[:])
                        nc.vector.tensor_mul(attn_sb[h][:, tq0:tq0 + TQH],
                                             oraw[:], inv_t[:])

                def emit_oproj_quarter(s, po_pool, ostg, wo_sb):
                    s0 = s * QT
                    for mg in range(0, MB, 4):
                        psums = []
                        for k in range(4):
                            pt = po_pool.tile([P, QT], F32, tag="po",
                                              name=f"po{s}_{mg}_{k}")
                            psums.append(pt)
                        for k in range(4):
                            m = mg + k
                            for h in range(QH):
                                lhsT = wo_sb[:, h * HID + m * P:
                                             h * HID + (m + 1) * P]
                                nc.tensor.matmul(
                                    psums[k][:], lhsT=lhsT,
                                    rhs=attn_sb[h][:, s0:s0 + QT],
                                    start=(h == 0), stop=(h == QH - 1))
                        for k in range(4):
                            m = mg + k
                            ob = ostg.tile([P, QT], BF16, tag="ob", name="ob")
                            # DVE-only evac: keeps the Act queue free for the
                            # interleaved attention exps and the Pool queue
                            # free for the RSs.  Quarter 0's DMAs stay off the
                            # scalar queue too (a(3)'s exps follow them);
                            # later quarters have no attention behind them.
                            nc.vector.tensor_copy(ob[:], psums[k][:])
                            nc.sync.dma_start(
                                partial[s][m * P:(m + 1) * P, :], ob[:])
                    if not _NO_RS:
                        nc.gpsimd.collective_compute(
                            "ReduceScatter",
                            mybir.AluOpType.add,
                            ins=[partial[s][:]],
                            outs=[rs_out[s][:]],
                            replica_groups=[list(range(N_CORES))],
                        )

                # ---------------- interleaved emission ----------------
                emit_qkv_block(0)
                emit_qkv_block(1)
                if phases >= 2:
                    emit_attn_block(0)
                emit_qkv_block(2)
                if phases >= 2:
                    emit_attn_block(1)
                emit_qkv_block(3)
                ph1.close()

                if phases >= 3:
                    po_pool = body.enter_context(
                        tc.tile_pool(name="oproj_psum", bufs=4, space="PSUM"))
                    ostg = body.enter_context(tc.tile_pool(name="ostg", bufs=4))
                    wo_pool = body.enter_context(tc.tile_pool(name="wo", bufs=1))
                    # o_proj weights: [local headcol 512, HID] -> [p, h, m]
                    wo_sb = wo_pool.tile([P, QH * HID], BF16, tag="wo")
                    wo_src = woT.rearrange("(h p) m -> p h m", p=P)
                    wo_dst = wo_sb.rearrange("p (h m) -> p h m", m=HID)
                    for h in range(QH):
                        nc.sync.dma_start(wo_dst[:, h:h + 1, :],
                                          wo_src[:, h:h + 1, :])

                if phases >= 2:
                    emit_attn_block(2)
                if phases >= 3:
                    emit_oproj_quarter(0, po_pool, ostg, wo_sb)
                if phases >= 2:
                    emit_attn_block(3)
                if phases >= 3:
                    emit_oproj_quarter(1, po_pool, ostg, wo_sb)
                    emit_oproj_quarter(2, po_pool, ostg, wo_sb)
                    emit_oproj_quarter(3, po_pool, ostg, wo_sb)
                    # all output copies at the end: a DMA waiting on an RS
                    # holds its queue's SEQ, so don't put one in front of
                    # work the PE still needs
                    for s in range(nqt):
                        nc.gpsimd.dma_start(outp[:, s * QT:(s + 1) * QT],
                                            rs_out[s][:])

    nc.compile()
    return nc


def make_inputs(positions, hidden_states, w_qkv, w_o):
    """Host-side shard + relayout.  Returns per-core input maps."""
    half = D // 2
    inv_freq = 1.0 / (1e6 ** (np.arange(0, half, dtype=np.float32) / half))
    freqs = positions.astype(np.float32)[:, None] * inv_freq[None, :]
    cosT = np.cos(freqs).T.astype(np.float32)      # [64, T]
    sinT = np.sin(freqs).T.astype(np.float32)
    cos2 = np.ascontiguousarray(np.concatenate([cosT, cosT], axis=0))
    sin2 = np.ascontiguousarray(np.concatenate([-sinT, sinT], axis=0))

    ii = np.arange(P)
    maskd = np.where(ii[None, :] >= ii[:, None], 0.0, NEG).astype(np.float32)

    hiddenT = np.ascontiguousarray(hidden_states.T).astype(bf16)

    q_size = 32 * D
    in_maps = []
    for i in range(N_CORES):
        rows = np.concatenate([
            w_qkv[QH * P * i:QH * P * (i + 1)],                      # 4 q heads
            w_qkv[q_size + P * i:q_size + P * (i + 1)],              # k head
            w_qkv[q_size + 8 * D + P * i:q_size + 8 * D + P * (i + 1)],  # v head
        ], axis=0)
        wqkvT_i = np.ascontiguousarray(rows.T).astype(bf16)
        # o_proj: my 512 head-columns of w_o, transposed -> [512, 4096]
        woT_i = np.ascontiguousarray(
            w_o[:, QH * P * i:QH * P * (i + 1)].T).astype(bf16)
        in_maps.append({
            "hiddenT": hiddenT,
            "wqkvT": wqkvT_i,
            "woT": woT_i,
            "cos2": cos2,
            "sin2": sin2,
            "maskd": maskd,
        })
    return in_maps


def assemble(results, t_len=2048):
    final = np.empty((t_len, N_CORES * QH * P), dtype=np.float32)
    for i in range(N_CORES):
        final[:, QH * P * i:QH * P * (i + 1)] = \
            results[i]["outp"].astype(np.float32).T
    return final


def kernel(positions, hidden_states, w_qkv, w_o):
    positions = np.asarray(positions)
    hidden_states = np.asarray(hidden_states, dtype=np.float32)
    w_qkv = np.asarray(w_qkv, dtype=np.float32)
    w_o = np.asarray(w_o, dtype=np.float32)
    t_len = hidden_states.shape[0]

    nc = build_nc(t_len)
    in_maps = make_inputs(positions, hidden_states, w_qkv, w_o)
    res = run_bass_kernel_spmd(nc, in_maps, list(range(N_CORES)))
    return assemble(res.results, t_len)


# revision 46
# speedup vs baseline: 2.1497x; 2.1497x over previous
"""Mixtral-style GQA attention block, tensor-parallel over 8 NeuronCores.

Sharding: core i owns q heads 4i..4i+3 and kv head i (GQA group == 4, so the
kv head's whole group lives on one core).  w_qkv is column-sharded by head.
o_proj is token-sharded via contraction-split: each core multiplies its own
attention outputs (kept resident in SBUF) against its 512-column slice of
w_o, producing a full-width partial [4096, T]; a bf16 ReduceScatter per
token-quarter sums the partials and leaves core i with the 512-row output
slice it owns.

Emission order interleaves the three stages so the PE never waits on a
phase boundary and the ReduceScatter chain drains under attention compute:
  qkv(0) qkv(1) attn(0) qkv(2) attn(1) qkv(3)
  attn(2) oproj(0)+RS0 attn(3) oproj(1)+RS1 oproj(2)+RS2 oproj(3)+RS3

All matmuls run in bf16 (fp32 PSUM accumulation); softmax runs in fp32
without max-subtraction (scores are ~N(0,1) by construction, exp cannot
overflow).
"""

import os
import numpy as np
import ml_dtypes
from contextlib import ExitStack

import concourse.bass as bass
import concourse.bass_isa as bass_isa
import concourse.mybir as mybir
import concourse.tile as tile
from concourse import bacc
from concourse.bass_utils import run_bass_kernel_spmd

_NO_RS = bool(int(os.environ.get("K_NO_RS", "0")))   # ablation only

P = 128
HID = 4096
D = 128
QH = 4                      # local q heads per core
NB = 6                      # projection M-blocks: q0..q3, k, v
KC = HID // P               # contraction chunks over hidden dim
N_CORES = 8
SCALE = float(D) ** -0.5
NEG = -1.0e30

dt = mybir.dt
bf16 = ml_dtypes.bfloat16

F32 = dt.float32
BF16 = dt.bfloat16


def build_nc(t_len=2048, phases=3, reps=1):
    TCH = t_len // P            # token chunks
    TB = 512                    # qkv projection token-block width
    TQH = max(t_len // 4, P)    # attention tq-block width
    R = min(512, TQH)           # psum accumulation region width
    NR = TQH // R
    WQ = NB * P                 # 768
    WO = QH * P                 # 512
    QT = min(512, t_len)        # o_proj token-tile width
    QT = 512
    nqt = t_len // QT
    MB = HID // P               # o_proj hidden-out blocks (32)
    n_tq = t_len // TQH

    nc = bacc.Bacc("TRN2", target_bir_lowering=False, debug=False,
                   num_devices=N_CORES)

    hiddenT = nc.dram_tensor("hiddenT", [HID, t_len], BF16, kind="ExternalInput").ap()
    wqkvT = nc.dram_tensor("wqkvT", [HID, WQ], BF16, kind="ExternalInput").ap()
    woT = nc.dram_tensor("woT", [WO, HID], BF16, kind="ExternalInput").ap()
    cos2 = nc.dram_tensor("cos2", [P, t_len], F32, kind="ExternalInput").ap()
    sin2 = nc.dram_tensor("sin2", [P, t_len], F32, kind="ExternalInput").ap()
    maskd = nc.dram_tensor("maskd", [P, P], F32, kind="ExternalInput").ap()
    outp = nc.dram_tensor("outp", [WO, t_len], BF16, kind="ExternalOutput").ap()

    with tile.TileContext(nc) as tc:
        with ExitStack() as whole:
            persist = whole.enter_context(tc.tile_pool(name="persist", bufs=1))
            dram = whole.enter_context(tc.tile_pool(name="dram", bufs=1, space="DRAM"))

            # ---- constants ----
            mask_sb = persist.tile([P, P], F32, tag="mask")
            # full ones matrix: the l matmul then leaves the row-sum
            # broadcast across all 128 output partitions directly
            ones_sb = persist.tile([P, P], BF16, tag="ones")
            nc.gpsimd.dma_start(mask_sb[:], maskd[:])
            nc.vector.memset(ones_sb[:], 1.0)

            # ---- persistent activations ----
            qk_sb = [persist.tile([P, t_len], BF16, tag=f"qk{mb}", name=f"qk{mb}")
                     for mb in range(5)]
            v_sb = persist.tile([P, TCH * P], BF16, tag="v", name="v_sb")
            # attention outputs overwrite the q tiles in place: block `half`
            # of head h reads qT[h][:, tq0:tq0+TQH] exactly once (its score
            # matmuls), after which the region is dead -- so the normalized
            # attention output for that block can alias it.  o_proj then
            # reads its rhs straight from these tiles.
            attn_sb = qk_sb[:QH]

            for rep in range(reps):
              partial = [dram.tile([HID, QT], BF16, tag=f"partial{rep}_{s}",
                                   name=f"partial{rep}_{s}")
                         for s in range(nqt)]
              rs_out = [dram.tile([WO, QT], BF16, tag=f"rs{rep}_{s}",
                                  name=f"rs{rep}_{s}")
                        for s in range(nqt)]

              with ExitStack() as body:
                # ---------- attention pools (live for the whole rep) ----------
                p_pool = body.enter_context(tc.tile_pool(name="pstrips", bufs=20))
                st_pool = body.enter_context(
                    tc.tile_pool(name="st_psum", bufs=2, space="PSUM"))
                out_ps_pool = body.enter_context(
                    tc.tile_pool(name="attn_out_psum", bufs=1, space="PSUM"))
                l_ps_pool = body.enter_context(
                    tc.tile_pool(name="l_psum", bufs=1, space="PSUM"))
                misc = body.enter_context(tc.tile_pool(name="attn_misc", bufs=2))

                # ---------- phase-1 pools (closed after last qkv block) ------
                ph1 = ExitStack()
                consts = ph1.enter_context(tc.tile_pool(name="consts", bufs=1))
                hid_pool = ph1.enter_context(tc.tile_pool(name="hid", bufs=2))
                wq_pool = ph1.enter_context(tc.tile_pool(name="wq", bufs=1))
                ps_pool = ph1.enter_context(
                    tc.tile_pool(name="proj_psum", bufs=4, space="PSUM"))
                stage = ph1.enter_context(tc.tile_pool(name="stage", bufs=6))
                vpool = ph1.enter_context(tc.tile_pool(name="vstage", bufs=2))

                cos2_sb = consts.tile([P, t_len], F32, tag="cos2")
                sin2_sb = consts.tile([P, t_len], F32, tag="sin2")
                nc.gpsimd.dma_start(cos2_sb[:], cos2[:])
                nc.gpsimd.dma_start(sin2_sb[:], sin2[:])

                # cache all of wqkvT in SBUF: chunk c at [:, c*WQ:(c+1)*WQ]
                wqkv_sb = wq_pool.tile([P, KC * WQ], BF16, tag="wqkv")
                wqkv_src = wqkvT.rearrange("(c p) w -> p c w", p=P)
                wqkv_dst = wqkv_sb.rearrange("p (c w) -> p c w", w=WQ)
                # fine-grained leading chunks so the first matmuls start
                # early; the very first matmul (pair (4,5), c=0) only reads
                # the k/v columns of chunk 0, so those 256 columns go first
                # first chunk rides the sync queue ahead of everything (the
                # Act queue head holds the hoisted activation-table load);
                # the rest streams on Act in small batches to bound
                # head-of-line blocking
                nc.sync.dma_start(wqkv_dst[:, 0:1, 4 * P:6 * P],
                                  wqkv_src[:, 0:1, 4 * P:6 * P])
                # q columns of chunk 0 aren't needed until the second pair
                # (~15us in) -- keep the sync queue free for hid chunk 0
                nc.scalar.dma_start(wqkv_dst[:, 0:1, 0:4 * P],
                                    wqkv_src[:, 0:1, 0:4 * P])
                for a, b in [(1, 2), (2, 3), (3, 4)] + \
                            [(c, c + 2) for c in range(4, KC, 2)]:
                    nc.scalar.dma_start(wqkv_dst[:, a:b, :],
                                        wqkv_src[:, a:b, :])

                hid_src = hiddenT.rearrange("(c p) t -> p c t", p=P)

                def emit_qkv_block(tb):
                    ta = tb * TB
                    hid_b = hid_pool.tile([P, KC * TB], BF16, tag="hidb",
                                          name=f"hid{tb}")
                    hb_dst = hid_b.rearrange("p (c t) -> p c t", t=TB)
                    if tb == 0:
                        batches = [(0, 1), (1, 2), (2, 3), (3, 4), (4, 8),
                                   (8, 16), (16, 24), (24, 32)]
                    else:
                        batches = [(c, c + 8) for c in range(0, KC, 8)]
                    for a, b in batches:
                        nc.sync.dma_start(hb_dst[:, a:b, :],
                                          hid_src[:, a:b, ta:ta + TB])
                    for pair, mbs in enumerate([(4, 5), (0, 1), (2, 3)]):
                        psums = []
                        for u in range(2):
                            pt = ps_pool.tile([P, TB], F32, tag="pj",
                                              name=f"pj{tb}_{pair}_{u}")
                            psums.append(pt)
                        for c in range(KC):
                            for u in range(2):
                                mb = mbs[u]
                                lhsT = wqkv_sb[:, c * WQ + mb * P:
                                               c * WQ + (mb + 1) * P]
                                nc.tensor.matmul(
                                    psums[u][:], lhsT=lhsT,
                                    rhs=hid_b[:, c * TB:(c + 1) * TB],
                                    start=(c == 0), stop=(c == KC - 1))
                        for u in range(2):
                            mb = mbs[u]
                            if mb < 5:
                                # rope: qk[d] = raw[d]*cos2[d]
                                #             + raw[(d+64)%128]*sin2[d]
                                raw = stage.tile([P, TB], F32, tag="stg",
                                                 name="raw")
                                nc.vector.tensor_copy(raw[:], psums[u][:])
                                rot = stage.tile([P, TB], F32, tag="stg",
                                                 name="rot")
                                nc.gpsimd.dma_start(rot[0:64, :], raw[64:128, :])
                                nc.gpsimd.dma_start(rot[64:128, :], raw[0:64, :])
                                t1 = stage.tile([P, TB], F32, tag="stg",
                                                name="t1")
                                nc.vector.tensor_mul(t1[:], raw[:],
                                                     cos2_sb[:, ta:ta + TB])
                                t2 = stage.tile([P, TB], F32, tag="stg",
                                                name="t2")
                                nc.vector.tensor_mul(t2[:], rot[:],
                                                     sin2_sb[:, ta:ta + TB])
                                nc.vector.tensor_add(qk_sb[mb][:, ta:ta + TB],
                                                     t1[:], t2[:])
                            else:
                                # v: evacuate bf16 [d, t], DMA-transpose each
                                # [d, tk] chunk into [tk, d]
                                vstg = vpool.tile([P, TB], BF16, tag="vstg",
                                                  name="vstg")
                                nc.vector.tensor_copy(vstg[:], psums[u][:])
                                for ct in range(TB // P):
                                    gc = tb * (TB // P) + ct
                                    nc.sync.dma_start_transpose(
                                        v_sb[:, gc * P:(gc + 1) * P],
                                        vstg[:, ct * P:(ct + 1) * P])

                kT = qk_sb[4]

                def emit_attn_block(half, add_eng):
                    for h in range(QH):
                        qT = qk_sb[h]
                        tq0 = TQH * half
                        ncv = (tq0 + TQH) // P    # contributing tk chunks
                        # ---- pass A: scores + exp -> P strips ----
                        strips = []
                        for c in range(ncv):
                            off = max(tq0, P * c)
                            w = tq0 + TQH - off
                            st = st_pool.tile([P, TQH], F32, tag="st",
                                              name=f"st{h}_{half}_{c}")
                            for s0 in range(0, w, 512):
                                s1 = min(s0 + 512, w)
                                nc.tensor.matmul(
                                    st[:, s0:s1],
                                    lhsT=kT[:, c * P:(c + 1) * P],
                                    rhs=qT[:, off + s0:off + s1],
                                    start=True, stop=True)
                            if P * c >= tq0:
                                # strip starts on the diagonal: mask tq<tk
                                nc.vector.tensor_add(
                                    st[:, 0:P], st[:, 0:P], mask_sb[:])
                            pt = p_pool.tile([P, TQH], BF16, tag="p",
                                             name=f"p{h}_{half}_{c}")
                            nc.scalar.activation(
                                pt[:, 0:w], st[:, 0:w],
                                mybir.ActivationFunctionType.Exp, scale=SCALE)
                            strips.append((pt, off, w))
                        # ---- strip pre-sum: l needs only sum_c sum_tk pt_c,
                        # so accumulate the strips elementwise on an idle
                        # engine (Pool / DVE) and charge the PE one 512-row
                        # matmul instead of re-streaming every strip ----
                        sacc = misc.tile([P, TQH], BF16, tag="sacc",
                                         name=f"sacc{half}_{h}")
                        pt0, off0, w0 = strips[0]
                        add_eng.tensor_copy(sacc[:, off0 - tq0:], pt0[:, 0:w0])
                        for c in range(1, len(strips)):
                            pt, off, w = strips[c]
                            a0 = off - tq0
                            add_eng.tensor_add(sacc[:, a0:], sacc[:, a0:],
                                               pt[:, 0:w])
                        # ---- pass B: PV region-wise ----
                        out_ps = out_ps_pool.tile([P, TQH], F32, tag="op",
                                                  name="out_ps")
                        l_ps = l_ps_pool.tile([P, TQH], F32, tag="lp",
                                              name="l_ps")
                        for s0 in range(0, TQH, 512):
                            s1 = min(s0 + 512, TQH)
                            nc.tensor.matmul(l_ps[:, s0:s1], lhsT=ones_sb[:],
                                             rhs=sacc[:, s0:s1],
                                             start=True, stop=True)
                        for r in range(NR):
                            r0 = tq0 + R * r
                            cmax = (r0 + R - 1) // P
                            for c in range(cmax + 1):
                                pt, off, w = strips[c]
                                a = max(0, r0 - off)
                                b = max(0, off - r0)
                                wr = min(off + w, r0 + R) - max(off, r0)
                                dst0 = R * r + b
                                for s0 in range(0, wr, 512):
                                    s1 = min(s0 + 512, wr)
                                    nc.tensor.matmul(
                                        out_ps[:, dst0 + s0:dst0 + s1],
                                        lhsT=v_sb[:, c * P:(c + 1) * P],
                                        rhs=pt[:, a + s0:a + s1],
                                        start=(c == 0), stop=(c == cmax))
                        # ---- epilogue: normalize by row-sums ----
                        oraw = misc.tile([P, TQH], F32, tag="oraw", name="oraw")
                        nc.vector.tensor_copy(oraw[:], out_ps[:])
                        inv_t = misc.tile([P, TQH], F32, tag="inv", name="inv_t")
                        nc.vector.reciprocal(inv_t[:], l_ps[:])
                        nc.vector.tensor_mul(attn_sb[h][:, tq0:tq0 + TQH],
                                             oraw[:], inv_t[:])

                def emit_oproj_quarter(s, po_pool, ostg, wo_sb):
                    s0 = s * QT
                    for mg in range(0, MB, 4):
                        psums = []
                        for k in range(4):
                            pt = po_pool.tile([P, QT], F32, tag="po",
                                              name=f"po{s}_{mg}_{k}")
                            psums.append(pt)
                        for k in range(4):
                            m = mg + k
                            for h in range(QH):
                                lhsT = wo_sb[:, h * HID + m * P:
                                             h * HID + (m + 1) * P]
                                nc.tensor.matmul(
                                    psums[k][:], lhsT=lhsT,
                                    rhs=attn_sb[h][:, s0:s0 + QT],
                                    start=(h == 0), stop=(h == QH - 1))
                        for k in range(4):
                            m = mg + k
                            ob = ostg.tile([P, QT], BF16, tag="ob", name="ob")
                            # DVE-only evac: keeps the Act queue free for the
                            # interleaved attention exps and the Pool queue
                            # free for the RSs.  Quarter 0's DMAs stay off the
                            # scalar queue too (a(3)'s exps follow them);
                            # later quarters have no attention behind them.
                            nc.vector.tensor_copy(ob[:], psums[k][:])
                            nc.sync.dma_start(
                                partial[s][m * P:(m + 1) * P, :], ob[:])
                    if not _NO_RS:
                        nc.gpsimd.collective_compute(
                            "ReduceScatter",
                            mybir.AluOpType.add,
                            ins=[partial[s][:]],
                            outs=[rs_out[s][:]],
                            replica_groups=[list(range(N_CORES))],
                        )

                # ---------------- interleaved emission ----------------
                emit_qkv_block(0)
                emit_qkv_block(1)
                if phases >= 2:
                    emit_attn_block(0, nc.gpsimd)
                emit_qkv_block(2)
                if phases >= 2:
                    emit_attn_block(1, nc.gpsimd)
                emit_qkv_block(3)
                ph1.close()

                if phases >= 3:
                    po_pool = body.enter_context(
                        tc.tile_pool(name="oproj_psum", bufs=4, space="PSUM"))
                    ostg = body.enter_context(tc.tile_pool(name="ostg", bufs=4))
                    wo_pool = body.enter_context(tc.tile_pool(name="wo", bufs=1))
                    # o_proj weights: [local headcol 512, HID] -> [p, h, m]
                    wo_sb = wo_pool.tile([P, QH * HID], BF16, tag="wo")
                    wo_src = woT.rearrange("(h p) m -> p h m", p=P)
                    wo_dst = wo_sb.rearrange("p (h m) -> p h m", m=HID)
                    for h in range(QH):
                        for m0 in range(0, HID, HID // 2):
                            nc.sync.dma_start(
                                wo_dst[:, h:h + 1, m0:m0 + HID // 2],
                                wo_src[:, h:h + 1, m0:m0 + HID // 2])

                if phases >= 2:
                    # a(2) runs before RS0 exists: Pool queue is safe; a(3)
                    # interleaves with the collectives, so its strip-sums go
                    # to the DVE instead
                    emit_attn_block(2, nc.gpsimd)
                if phases >= 3:
                    emit_oproj_quarter(0, po_pool, ostg, wo_sb)
                if phases >= 2:
                    emit_attn_block(3, nc.vector)
                if phases >= 3:
                    emit_oproj_quarter(1, po_pool, ostg, wo_sb)
                    emit_oproj_quarter(2, po_pool, ostg, wo_sb)
                    emit_oproj_quarter(3, po_pool, ostg, wo_sb)
                    # all output copies at the end: a DMA waiting on an RS
                    # holds its queue's SEQ, so don't put one in front of
                    # work the PE still needs
                    for s in range(nqt):
                        nc.gpsimd.dma_start(outp[:, s * QT:(s + 1) * QT],
                                            rs_out[s][:])

    nc.compile()
    return nc


def make_inputs(positions, hidden_states, w_qkv, w_o):
    """Host-side shard + relayout.  Returns per-core input maps."""
    half = D // 2
    inv_freq = 1.0 / (1e6 ** (np.arange(0, half, dtype=np.float32) / half))
    freqs = positions.astype(np.float32)[:, None] * inv_freq[None, :]
    cosT = np.cos(freqs).T.astype(np.float32)      # [64, T]
    sinT = np.sin(freqs).T.astype(np.float32)
    cos2 = np.ascontiguousarray(np.concatenate([cosT, cosT], axis=0))
    sin2 = np.ascontiguousarray(np.concatenate([-sinT, sinT], axis=0))

    ii = np.arange(P)
    maskd = np.where(ii[None, :] >= ii[:, None], 0.0, NEG).astype(np.float32)

    hiddenT = np.ascontiguousarray(hidden_states.T).astype(bf16)

    q_size = 32 * D
    in_maps = []
    for i in range(N_CORES):
        rows = np.concatenate([
            w_qkv[QH * P * i:QH * P * (i + 1)],                      # 4 q heads
            w_qkv[q_size + P * i:q_size + P * (i + 1)],              # k head
            w_qkv[q_size + 8 * D + P * i:q_size + 8 * D + P * (i + 1)],  # v head
        ], axis=0)
        wqkvT_i = np.ascontiguousarray(rows.T).astype(bf16)
        # o_proj: my 512 head-columns of w_o, transposed -> [512, 4096]
        woT_i = np.ascontiguousarray(
            w_o[:, QH * P * i:QH * P * (i + 1)].T).astype(bf16)
        in_maps.append({
            "hiddenT": hiddenT,
            "wqkvT": wqkvT_i,
            "woT": woT_i,
            "cos2": cos2,
            "sin2": sin2,
            "maskd": maskd,
        })
    return in_maps


def assemble(results, t_len=2048):
    final = np.empty((t_len, N_CORES * QH * P), dtype=np.float32)
    for i in range(N_CORES):
        final[:, QH * P * i:QH * P * (i + 1)] = \
            results[i]["outp"].astype(np.float32).T
    return final


def kernel(positions, hidden_states, w_qkv, w_o):
    positions = np.asarray(positions)
    hidden_states = np.asarray(hidden_states, dtype=np.float32)
    w_qkv = np.asarray(w_qkv, dtype=np.float32)
    w_o = np.asarray(w_o, dtype=np.float32)
    t_len = hidden_states.shape[0]

    nc = build_nc(t_len)
    in_maps = make_inputs(positions, hidden_states, w_qkv, w_o)
    res = run_bass_kernel_spmd(nc, in_maps, list(range(N_CORES)))
    return assemble(res.results, t_len)
